# revision 4
# baseline (speedup 1.0000x reference)
"""Trainium2 Bass kernel for the KBLN scoring model.

Computes, for full inputs:
    score_l = (emb_e[e1] * emb_rel[rel]) @ emb_e.T                       (B, E)
    phi     = exp(-((lit[e1][:,None,:] - lit[None,:,:]) - c)^2 / var)    (B, E, L)
    score_n = einsum('bel,bl->be', phi, nf_weights[rel])
    out     = sigmoid(score_l + score_n)

Reformulation
-------------
With alpha[b,l] = (lit[e1[b],l] - 0.5 - c[l]) / sqrt(var[l]),
     beta[e,l]  = (lit[e,l]    - 0.5)        / sqrt(var[l]),
     g[l]       = -c[l] / sqrt(var[l]):

    phi = exp(-(alpha - beta)^2)
        = exp(-alpha^2) * exp(-(beta-g)^2 + g^2) * exp(2*(alpha-g)*beta)

The cross term x = 2*(alpha-g)*beta satisfies |x| <= 1, so a degree-3
Chebyshev (near-minimax) polynomial of exp(x) is accurate to ~6e-3.  That
turns score_n into a single matmul with contraction 4*64 = 256, fused with
the 200 emb dims of score_l into one (256 x 456) @ (456 x E_shard) matmul
per core:

    A[b,(k,l)]  = w[b,l] * exp(-alpha^2) * cheb_k * (2*(alpha-g))^k   (host)
    Bt[(k,l),e] = exp(-(beta-g)^2 + g^2) * beta^k                     (host)

Everything the device touches is bf16 (tolerance is 2e-2; measured rel err
of this scheme is ~5e-3).  All rhs factor tiles are precomputed on host
(they depend only on lit/c/var/emb_e), so the device program is purely:
DMA-in -> 32 accumulating matmuls -> 8 sigmoids -> DMA-out.

All DRAM inputs are pre-packed on host so that every DMA descriptor is a
contiguous multi-KB run per SBUF partition (slice-major layout), and the
entity slices are separate tensors so the tensor engine can start on
slice 0 while later slices stream in.

Sharding: entities (E=15000) split evenly across 8 cores (1875 each);
batch side replicated; outputs concatenated on host.
"""

import sys

import numpy as np

for _p in ("/opt/trn_rl_repo", "/root/.axon_site/_ro/trn_rl_repo"):
    if _p not in sys.path:
        sys.path.append(_p)

import concourse.bass as bass
import concourse.bacc as bacc
import concourse.mybir as mybir
from concourse import tile
from concourse import bass_utils

B, E, R, D, L = 256, 15000, 237, 200, 64
NCORES = 8
ES = E // NCORES          # 1875 entities per core
KT = 4                    # polynomial terms k = 0..3
KTOT = KT * L + D         # 456 contraction rows
F32 = mybir.dt.float32
BF16 = mybir.dt.bfloat16
# degree-3 Chebyshev monomial coefficients of e^x on [-1,1]
CHEB = (0.99457054, 0.99730766, 0.54299068, 0.1773474)
S_SLICES = [(0, 512), (512, 512), (1024, 512), (1536, 339)]

TRACE = False             # test.py sets True to collect an NTFF profile
LAST = None               # last BassKernelResults (for test.py)

_PROG = None              # cached Bass program


def _build_program():
    nc = bacc.Bacc("TRN2", target_bir_lowering=False, debug=False)

    # slice-major packed inputs: rts{s}[p, j*nsz+n] = Bt[j*128+p, n0+n]
    rts_d = [
        nc.dram_tensor(f"rts{si}", [128, 3 * nsz], BF16, kind="ExternalInput")
        for si, (n0, nsz) in enumerate(S_SLICES)
    ]
    r3s_d = [
        nc.dram_tensor(f"r3s{si}", [72, nsz], BF16, kind="ExternalInput")
        for si, (n0, nsz) in enumerate(S_SLICES)
    ]
    lhp_d = nc.dram_tensor("lhp", [128, 3 * B], BF16, kind="ExternalInput")
    lh3_d = nc.dram_tensor("lh3", [72, B], BF16, kind="ExternalInput")
    out_d = nc.dram_tensor("out", [B, ES], BF16, kind="ExternalOutput")

    AF = mybir.ActivationFunctionType

    with tile.TileContext(nc) as tc:
        with (
            tc.tile_pool(name="persist", bufs=1) as pool,
            tc.tile_pool(name="psum", bufs=8, space="PSUM") as ppool,
            tc.tile_pool(name="outs", bufs=4) as opool,
        ):
            lhp = pool.tile([128, 3 * B], BF16)
            lh3 = pool.tile([128, B], BF16)
            rts = [
                pool.tile([128, 3 * nsz], BF16, name=f"rts{si}")
                for si, (n0, nsz) in enumerate(S_SLICES)
            ]
            r3t = [
                pool.tile([128, nsz], BF16, name=f"r3t{si}")
                for si, (n0, nsz) in enumerate(S_SLICES)
            ]

            # sync queue: lhs, then rhs slices 1..3
            nc.sync.dma_start(lhp[:, :], lhp_d[:, :])
            nc.sync.dma_start(lh3[:72, :], lh3_d[:, :])
            # scalar queue: rhs slice 0 (engine is free until sigmoids start)
            nc.scalar.dma_start(rts[0][:, :], rts_d[0][:, :])
            for si in (1, 2, 3):
                nc.sync.dma_start(rts[si][:, :], rts_d[si][:, :])
            # gpsimd/SWDGE: the small 72-row tile slices
            for si in range(4):
                nc.gpsimd.dma_start(r3t[si][:72, :], r3s_d[si][:, :])

            for m in range(2):
                ms = np.s_[m * 128 : (m + 1) * 128]
                for si, (n0, nsz) in enumerate(S_SLICES):
                    ps = ppool.tile([128, 512], F32, name="ps")
                    for j in range(4):
                        if j < 3:
                            rhs = rts[si][:, j * nsz : (j + 1) * nsz]
                            lhs = lhp[:, j * B + m * 128 : j * B + (m + 1) * 128]
                        else:
                            rhs = r3t[si][:72, :nsz]
                            lhs = lh3[:72, m * 128 : (m + 1) * 128]
                        nc.tensor.matmul(
                            ps[:, :nsz], lhs, rhs, start=(j == 0), stop=(j == 3)
                        )
                    ob = opool.tile([128, 512], BF16, name="ob")
                    nc.scalar.activation(ob[:, :nsz], ps[:, :nsz], AF.Sigmoid)
                    eng = nc.gpsimd if m == 0 else nc.sync
                    eng.dma_start(out_d[ms, n0 : n0 + nsz], ob[:, :nsz])

    nc.compile()
    return nc


def _host_prep(emb_e, emb_rel, nf_weights, lit, c, var, e1, rel):
    import ml_dtypes

    bf = ml_dtypes.bfloat16
    e1 = np.asarray(e1).astype(np.int64)
    rel = np.asarray(rel).astype(np.int64)
    lit64 = np.asarray(lit, np.float64)
    c64 = np.asarray(c, np.float64)
    var64 = np.asarray(var, np.float64)

    rsv = 1.0 / np.sqrt(var64)                      # (L,)
    g = -c64 * rsv

    # ---- lhs side (batch): A[b, k*64+l] and emb rows
    P = lit64[e1]                                   # (B, L)
    w = np.asarray(nf_weights, np.float64)[rel]     # (B, L)
    amg = (P - 0.5) * rsv                           # alpha - g
    alpha = amg + g
    u = np.exp(-(alpha**2)) * w                     # (B, L)
    t2 = 2.0 * amg
    lhsT = np.zeros((KTOT, B), bf)
    acc = u.copy()
    for k in range(KT):
        if k:
            acc = acc * t2
        lhsT[k * L : (k + 1) * L, :] = (CHEB[k] * acc).T.astype(bf)
    x = np.asarray(emb_e, np.float64)[e1] * np.asarray(emb_rel, np.float64)[rel]
    lhsT[KT * L :, :] = x.T.astype(bf)
    lhp = np.ascontiguousarray(
        lhsT[: 3 * 128].reshape(3, 128, B).transpose(1, 0, 2).reshape(128, 3 * B)
    )
    lh3 = np.ascontiguousarray(lhsT[3 * 128 :])

    # ---- rhs side (entities): Bt[k*64+l, e] = V * beta^k, then emb_e.T
    beta = (lit64 - 0.5) * rsv                      # (E, L)
    V = np.exp(beta * (2.0 * g - beta))             # (E, L)
    rhs = np.empty((KTOT, E), bf)
    accr = V.copy()
    for k in range(KT):
        if k:
            accr = accr * beta
        rhs[k * L : (k + 1) * L, :] = accr.T.astype(bf)
    rhs[KT * L :, :] = np.asarray(emb_e, np.float64).T.astype(bf)

    in_maps = []
    for ci in range(NCORES):
        lo = ci * ES
        Rj = rhs[: 3 * 128, lo : lo + ES].reshape(3, 128, ES)
        m = {"lhp": lhp, "lh3": lh3}
        for si, (n0, nsz) in enumerate(S_SLICES):
            m[f"rts{si}"] = np.ascontiguousarray(
                Rj[:, :, n0 : n0 + nsz].transpose(1, 0, 2).reshape(128, 3 * nsz)
            )
            m[f"r3s{si}"] = np.ascontiguousarray(
                rhs[3 * 128 :, lo + n0 : lo + n0 + nsz]
            )
        in_maps.append(m)
    return in_maps


def kernel(emb_e, emb_rel, nf_weights, lit, c, var, e1, rel):
    global _PROG, LAST
    if _PROG is None:
        _PROG = _build_program()
    in_maps = _host_prep(emb_e, emb_rel, nf_weights, lit, c, var, e1, rel)
    res = bass_utils.run_bass_kernel_spmd(
        _PROG, in_maps, core_ids=list(range(NCORES)), trace=TRACE
    )
    LAST = res
    return np.concatenate(
        [np.asarray(res.results[ci]["out"]).astype(np.float32) for ci in range(NCORES)],
        axis=1,
    )


# revision 6
# speedup vs baseline: 1.1445x; 1.1445x over previous
"""Trainium2 Bass kernel for the KBLN scoring model.

Computes, for full inputs:
    score_l = (emb_e[e1] * emb_rel[rel]) @ emb_e.T                       (B, E)
    phi     = exp(-((lit[e1][:,None,:] - lit[None,:,:]) - c)^2 / var)    (B, E, L)
    score_n = einsum('bel,bl->be', phi, nf_weights[rel])
    out     = sigmoid(score_l + score_n)

Reformulation
-------------
With alpha[b,l] = (lit[e1[b],l] - 0.5 - c[l]) / sqrt(var[l]),
     beta[e,l]  = (lit[e,l]    - 0.5)        / sqrt(var[l]),
     g[l]       = -c[l] / sqrt(var[l]):

    phi = exp(-(alpha - beta)^2)
        = exp(-alpha^2) * exp(-(beta-g)^2 + g^2) * exp(2*(alpha-g)*beta)

The cross term x = 2*(alpha-g)*beta satisfies |x| <= 1, so a degree-3
Chebyshev (near-minimax) polynomial of exp(x) is accurate to ~6e-3.  That
turns score_n into a single matmul with contraction 4*64 = 256, fused with
the 200 emb dims of score_l into one (256 x 456) @ (456 x E_shard) matmul
per core:

    A[b,(k,l)]  = w[b,l] * exp(-alpha^2) * cheb_k * (2*(alpha-g))^k   (host)
    Bt[(k,l),e] = exp(-(beta-g)^2 + g^2) * beta^k                     (host)

Everything the device touches is bf16 (tolerance is 2e-2; measured rel err
of this scheme is ~5e-3).  All rhs factor tiles are precomputed on host,
so the device program is purely: DMA-in -> 32 accumulating matmuls ->
8 sigmoids -> DMA-out.  It is written in raw Bass (no Tile framework) with
manual semaphores: Tile's generality costs ~6us of whole-range semaphore
clears in the postamble plus per-instruction clock traffic, which this
kernel doesn't need.

All DRAM inputs are pre-packed on host so that every DMA descriptor is a
contiguous multi-KB run per SBUF partition (slice-major layout), and the
entity slices are separate tensors so the tensor engine can start on
slice 0 while later slices stream in.  Input DMAs are spread over the
three DMA-capable queues (sync / scalar / gpsimd) to overlap transfers.

Sharding: entities (E=15000) split evenly across 8 cores (1875 each);
batch side replicated; outputs concatenated on host.
"""

import sys

import numpy as np

for _p in ("/opt/trn_rl_repo", "/root/.axon_site/_ro/trn_rl_repo"):
    if _p not in sys.path:
        sys.path.append(_p)

import concourse.bass as bass
import concourse.bacc as bacc
import concourse.mybir as mybir
from concourse import bass_utils

B, E, R, D, L = 256, 15000, 237, 200, 64
NCORES = 8
ES = E // NCORES          # 1875 entities per core
KT = 4                    # polynomial terms k = 0..3
KTOT = KT * L + D         # 456 contraction rows
F32 = mybir.dt.float32
BF16 = mybir.dt.bfloat16
# degree-3 Chebyshev monomial coefficients of e^x on [-1,1]
CHEB = (0.99457054, 0.99730766, 0.54299068, 0.1773474)
S_SLICES = [(0, 512), (512, 512), (1024, 512), (1536, 339)]

TRACE = False             # test.py sets True to collect an NTFF profile
LAST = None               # last BassKernelResults (for test.py)

_PROG = None              # cached Bass program


def _build_program():
    nc = bacc.Bacc("TRN2", target_bir_lowering=False, debug=False)

    AF = mybir.ActivationFunctionType

    # slice-major packed inputs: rts{s}[p, j*nsz+n] = Bt[j*128+p, n0+n]
    rts_d = [
        nc.dram_tensor(f"rts{si}", [128, 3 * nsz], BF16, kind="ExternalInput")
        for si, (n0, nsz) in enumerate(S_SLICES)
    ]
    r3s_d = [
        nc.dram_tensor(f"r3s{si}", [72, nsz], BF16, kind="ExternalInput")
        for si, (n0, nsz) in enumerate(S_SLICES)
    ]
    lhp_d = nc.dram_tensor("lhp", [128, 3 * B], BF16, kind="ExternalInput")
    lh3_d = nc.dram_tensor("lh3", [72, B], BF16, kind="ExternalInput")
    out_d = nc.dram_tensor("out", [B, ES], BF16, kind="ExternalOutput")

    rts = [
        nc.alloc_sbuf_tensor(f"rts_sb{si}", [128, 3 * nsz], BF16)
        for si, (n0, nsz) in enumerate(S_SLICES)
    ]
    r3t = [
        nc.alloc_sbuf_tensor(f"r3t_sb{si}", [72, nsz], BF16)
        for si, (n0, nsz) in enumerate(S_SLICES)
    ]
    lhp = nc.alloc_sbuf_tensor("lhp_sb", [128, 3 * B], BF16)
    lh3 = nc.alloc_sbuf_tensor("lh3_sb", [72, B], BF16)
    obs = [
        nc.alloc_sbuf_tensor(f"ob{g}", [128, S_SLICES[g % 4][1]], BF16)
        for g in range(8)
    ]
    pss = [
        nc.alloc_psum_tensor(f"ps{g}", [128, 512], F32) for g in range(8)
    ]

    s_lh = nc.alloc_semaphore("s_lh")
    s_lh3 = nc.alloc_semaphore("s_lh3")
    s_rts = [nc.alloc_semaphore(f"s_rts{si}") for si in range(4)]
    s_r3 = [nc.alloc_semaphore(f"s_r3{si}") for si in range(4)]
    s_mm = nc.alloc_semaphore("s_mm")
    s_sig = nc.alloc_semaphore("s_sig")
    s_out = nc.alloc_semaphore("s_out")

    with nc.Block("main") as blk:

        @blk.sync
        def _(eng):
            eng.dma_start(lhp[:, :], lhp_d[:, :]).then_inc(s_lh, 16)
            eng.dma_start(lh3[:, :], lh3_d[:, :]).then_inc(s_lh3, 16)
            eng.dma_start(rts[1][:, :], rts_d[1][:, :]).then_inc(s_rts[1], 16)
            eng.dma_start(rts[3][:, :], rts_d[3][:, :]).then_inc(s_rts[3], 16)
            # m1 outputs (groups 4..7)
            for si, (n0, nsz) in enumerate(S_SLICES):
                eng.wait_ge(s_sig, 5 + si)
                eng.dma_start(
                    out_d[128:256, n0 : n0 + nsz], obs[4 + si][:, :]
                ).then_inc(s_out, 16)

        @blk.scalar
        def _(eng):
            eng.dma_start(rts[0][:, :], rts_d[0][:, :]).then_inc(s_rts[0], 16)
            eng.dma_start(rts[2][:, :], rts_d[2][:, :]).then_inc(s_rts[2], 16)
            for g in range(8):
                nsz = S_SLICES[g % 4][1]
                eng.wait_ge(s_mm, g + 1)
                nc.scalar.activation(
                    obs[g][:, :], pss[g][:, :nsz], AF.Sigmoid
                ).then_inc(s_sig, 1)

        @blk.gpsimd
        def _(eng):
            for si in range(4):
                eng.dma_start(r3t[si][:, :], r3s_d[si][:, :]).then_inc(s_r3[si], 16)
            # m0 outputs (groups 0..3)
            for si, (n0, nsz) in enumerate(S_SLICES):
                eng.wait_ge(s_sig, 1 + si)
                eng.dma_start(
                    out_d[0:128, n0 : n0 + nsz], obs[si][:, :]
                ).then_inc(s_out, 16)

        @blk.tensor
        def _(eng):
            for m in range(2):
                for si, (n0, nsz) in enumerate(S_SLICES):
                    g = m * 4 + si
                    ps = pss[g]
                    if m == 0:
                        eng.wait_ge(s_rts[si], 16)
                        if si == 0:
                            eng.wait_ge(s_lh, 16)
                    for j in range(3):
                        nc.tensor.matmul(
                            ps[:, :nsz],
                            lhp[:, j * B + m * 128 : j * B + (m + 1) * 128],
                            rts[si][:, j * nsz : (j + 1) * nsz],
                            start=(j == 0),
                            stop=False,
                        )
                    if m == 0:
                        eng.wait_ge(s_r3[si], 16)
                        if si == 0:
                            eng.wait_ge(s_lh3, 16)
                    nc.tensor.matmul(
                        ps[:, :nsz],
                        lh3[:, m * 128 : (m + 1) * 128],
                        r3t[si][:, :],
                        start=False,
                        stop=True,
                    ).then_inc(s_mm, 1)

    nc.compile()
    return nc


def _host_prep(emb_e, emb_rel, nf_weights, lit, c, var, e1, rel):
    import ml_dtypes

    bf = ml_dtypes.bfloat16
    e1 = np.asarray(e1).astype(np.int64)
    rel = np.asarray(rel).astype(np.int64)
    lit64 = np.asarray(lit, np.float64)
    c64 = np.asarray(c, np.float64)
    var64 = np.asarray(var, np.float64)

    rsv = 1.0 / np.sqrt(var64)                      # (L,)
    g = -c64 * rsv

    # ---- lhs side (batch): A[b, k*64+l] and emb rows
    P = lit64[e1]                                   # (B, L)
    w = np.asarray(nf_weights, np.float64)[rel]     # (B, L)
    amg = (P - 0.5) * rsv                           # alpha - g
    alpha = amg + g
    u = np.exp(-(alpha**2)) * w                     # (B, L)
    t2 = 2.0 * amg
    lhsT = np.zeros((KTOT, B), bf)
    acc = u.copy()
    for k in range(KT):
        if k:
            acc = acc * t2
        lhsT[k * L : (k + 1) * L, :] = (CHEB[k] * acc).T.astype(bf)
    x = np.asarray(emb_e, np.float64)[e1] * np.asarray(emb_rel, np.float64)[rel]
    lhsT[KT * L :, :] = x.T.astype(bf)
    lhp = np.ascontiguousarray(
        lhsT[: 3 * 128].reshape(3, 128, B).transpose(1, 0, 2).reshape(128, 3 * B)
    )
    lh3 = np.ascontiguousarray(lhsT[3 * 128 :])

    # ---- rhs side (entities): Bt[k*64+l, e] = V * beta^k, then emb_e.T
    beta = (lit64 - 0.5) * rsv                      # (E, L)
    V = np.exp(beta * (2.0 * g - beta))             # (E, L)
    rhs = np.empty((KTOT, E), bf)
    accr = V.copy()
    for k in range(KT):
        if k:
            accr = accr * beta
        rhs[k * L : (k + 1) * L, :] = accr.T.astype(bf)
    rhs[KT * L :, :] = np.asarray(emb_e, np.float64).T.astype(bf)

    in_maps = []
    for ci in range(NCORES):
        lo = ci * ES
        Rj = rhs[: 3 * 128, lo : lo + ES].reshape(3, 128, ES)
        m = {"lhp": lhp, "lh3": lh3}
        for si, (n0, nsz) in enumerate(S_SLICES):
            m[f"rts{si}"] = np.ascontiguousarray(
                Rj[:, :, n0 : n0 + nsz].transpose(1, 0, 2).reshape(128, 3 * nsz)
            )
            m[f"r3s{si}"] = np.ascontiguousarray(
                rhs[3 * 128 :, lo + n0 : lo + n0 + nsz]
            )
        in_maps.append(m)
    return in_maps


def kernel(emb_e, emb_rel, nf_weights, lit, c, var, e1, rel):
    global _PROG, LAST
    if _PROG is None:
        _PROG = _build_program()
    in_maps = _host_prep(emb_e, emb_rel, nf_weights, lit, c, var, e1, rel)
    res = bass_utils.run_bass_kernel_spmd(
        _PROG, in_maps, core_ids=list(range(NCORES)), trace=TRACE
    )
    LAST = res
    return np.concatenate(
        [np.asarray(res.results[ci]["out"]).astype(np.float32) for ci in range(NCORES)],
        axis=1,
    )
